# revision 21
# baseline (speedup 1.0000x reference)
"""EntropyBottleneck forward (q_mode='noise') as a Trainium2 Bass kernel.

Math
----
reference computes, per channel c with tiny per-channel params (W_k, b_k, f_k):

    y    = x + noise
    v    = y flattened per channel
    L(v) = chain of FactorizeCell: u <- softplus(W_k) @ u + b_k,
           then u <- u + tanh(f_k) * tanh(u)   (for k < last)
    lower = L(v - 0.5); upper = L(v + 0.5)
    s     = -sign(lower + upper)
    lik   = max(|sigmoid(s*upper) - sigmoid(s*lower)|, 1e-9)

When every gate f_k == 0 (true for this module's initialization), the chain is
per-channel *affine*: L(v) = M_c * v + D_c, with M_c > 0 (product of softplus
matrices) and D_c foldable on the host from the (C,3,3)-at-most params.
Then with h = M_c/2 and t = M_c*y + D_c:

    lik = sigmoid(t + h) - sigmoid(t - h)  =  0.5*(tanh(a) - tanh(b)),
          a,b = (t +- h)/2               (sign trick folded; >= 0)

The 1e-9 lowerbound is a numeric no-op here (lik >= ~4e-3 always), so it is
dropped on device. With m = tanh(t/2) the exact identity

    lik = sinh(h) / (cosh(t) + cosh(h)) = tanh(h/2) * (1 - m^2) * (1 + eps*m^2),
    eps = (cosh(h)-1)/(1+cosh(h)) ~ 7e-5   (negligible: h = M/2 ~ 0.017)

reduces the device work to ONE tanh evaluation per element:

    y   = x + noise                            (vector, f32 in -> bf16 out)
    m   = tanh((M/2)*y + D/2)                  (scalar, fused affine, f32)
    u   = uint8((m*254)*m)                     (vector/gpsimd, one fused op;
                                                u8 convert rounds-to-nearest)

The kernel is HBM-bandwidth-bound (~410-430 GB/s/core across the two HWDGE
queues together), so outputs are stored in reduced precision:

  * y as bfloat16: relative rounding error <= 2^-8 at EVERY magnitude (bf16
    keeps the full f32 exponent range -- no subnormal blowup near zero).
  * lik as scaled uint8. W_k is channel-constant at init, so M (hence h and
    g0 = max lik = tanh(h/2)) is ONE global number, and lik/g0 = 1 - m^2
    in [~0.6, 1]: the uint8 code u = 254*(1 - m^2) + 0.5 has compile-time
    constants, spans ~[0, 102], and quantizes with <= 0.5 LSB error,
    i.e. <= ~0.7% of the smallest lik -- far under the 2e-2 gate.

Host reconstruction: lik = (254 - u) * (g0/254), y = float32(y_bf16).

Sharding: data-parallel over batch, one batch element per NeuronCore (8 cores).
Per-core tensor (192, 4096) is viewed as (384, 2048): row r holds half of
channel r//2, so each SBUF partition maps to exactly one channel and the
per-channel coefficients become per-partition scale/bias operands.

Schedule: loads are split across the two HWDGE FIFOs (x + the tiny param pack
on the SP FIFO via sync, noise on the ACT FIFO via scalar) so both rings
saturate; stores are balanced across the FIFOs behind the loads. The final
(128,1024) chunk is processed as two (128,512) halves so the compute tail
after the last load is short.
"""

import numpy as np

B, C, H, W = 8, 192, 64, 64
NCORES = 8
ROWS, COLS = 384, 2048  # (C, H*W) = (192, 4096) viewed as (384, 2048)
NT = ROWS // 128  # 3 row-tiles of 128 partitions
CH = 1024
# chunk list: (tile, col_start, width); last chunk split into two halves
CHUNKS = []
for _t in range(NT):
    for _h in range(COLS // CH):
        if _t == NT - 1 and _h == COLS // CH - 1:
            CHUNKS.append((_t, _h * CH, CH // 2))
            CHUNKS.append((_t, _h * CH + CH // 2, CH // 2))
        else:
            CHUNKS.append((_t, _h * CH, CH))
NCK = len(CHUNKS)  # 7

_CACHE: dict = {}


def _softplus64(x: np.ndarray) -> np.ndarray:
    x = x.astype(np.float64)
    return np.log1p(np.exp(-np.abs(x))) + np.maximum(x, 0.0)


def _fold_affine(ws, bs):
    """Compose the per-channel affine chain: L(v) = M*v + D. Returns (M, D) as (C,)."""
    M = np.ones((C, 1, 1), np.float64)
    D = np.zeros((C, 1, 1), np.float64)
    for Wk, bk in zip(ws, bs):
        spw = _softplus64(np.asarray(Wk))
        M = spw @ M
        D = spw @ D + np.asarray(bk, np.float64)
    return M[:, 0, 0], D[:, 0, 0]


def _numpy_fallback(x, noise, ws, bs, fs):
    """Exact replica of the reference chain for the general (gated) case."""
    x = np.asarray(x, np.float32)
    noise = np.asarray(noise, np.float32)
    y = x + noise
    v = y.transpose(1, 0, 2, 3).reshape(C, 1, -1).astype(np.float32)

    def logits(v):
        for i, (Wk, bk) in enumerate(zip(ws, bs)):
            spw = _softplus64(np.asarray(Wk)).astype(np.float32)
            v = np.einsum("coi,cin->con", spw, v) + np.asarray(bk, np.float32)
            if i < len(fs):
                v = v + np.tanh(np.asarray(fs[i], np.float32)) * np.tanh(v)
        return v

    lower = logits(v - 0.5)
    upper = logits(v + 0.5)
    sign = -np.sign(lower + upper)
    sig = lambda z: 1.0 / (1.0 + np.exp(-z, dtype=np.float32))
    lik = np.abs(sig(sign * upper) - sig(sign * lower))
    lik = np.maximum(lik, np.float32(1e-9))
    lik = lik.reshape(C, B, H, W).transpose(1, 0, 2, 3)
    return y, lik


def _build_program_raw():
    """Hand-scheduled per-engine instruction streams (see module docstring)."""
    import concourse.bacc as bacc
    import concourse.mybir as mybir

    f32 = mybir.dt.float32
    bf16 = mybir.dt.bfloat16
    u8 = mybir.dt.uint8
    nc = bacc.Bacc("TRN2", target_bir_lowering=False, debug=False,
                   num_devices=NCORES)

    x_d = nc.dram_tensor("x", [ROWS, COLS], f32, kind="ExternalInput")
    n_d = nc.dram_tensor("noise", [ROWS, COLS], f32, kind="ExternalInput")
    p_d = nc.dram_tensor("prm", [128, 2 * NT], f32, kind="ExternalInput")
    y_d = nc.dram_tensor("y", [ROWS, COLS], bf16, kind="ExternalOutput")
    l_d = nc.dram_tensor("lik", [ROWS, COLS], u8, kind="ExternalOutput")

    Tanh = mybir.ActivationFunctionType.Tanh
    op_add = mybir.AluOpType.add
    op_mult = mybir.AluOpType.mult

    prm = nc.alloc_sbuf_tensor("prms", [128, 2 * NT], f32)
    xts = [nc.alloc_sbuf_tensor(f"xt{t}", [128, COLS], f32) for t in range(NT)]
    nts = [nc.alloc_sbuf_tensor(f"nt{t}", [128, COLS], f32) for t in range(NT)]
    yts = [nc.alloc_sbuf_tensor(f"yt{t}", [128, COLS], bf16) for t in range(NT)]
    lts = [nc.alloc_sbuf_tensor(f"lt{t}", [128, COLS], u8) for t in range(NT)]
    mts = [nc.alloc_sbuf_tensor(f"mt{i}", [128, w], f32)
           for i, (_, _, w) in enumerate(CHUNKS)]

    # One semaphore per load chunk, waited only at the full total (+16 per
    # transfer from the 16 SDMA engines; prefix thresholds would be racy).
    ldg = [nc.alloc_semaphore(f"ld{i}") for i in range(NCK)]
    ldp = nc.alloc_semaphore("ldp")  # param pack
    va = nc.alloc_semaphore("va")    # vector adds (+1 each, engine-ordered)
    sa = nc.alloc_semaphore("sa")    # scalar tanh acts (+1 each)
    vtp = nc.alloc_semaphore("vtp")  # scalar quantizations (chunks 0..2)
    vtv = nc.alloc_semaphore("vtv")  # vector quantizations (chunks 3..6)
    st = nc.alloc_semaphore("st")    # all store completions
    n_stores = NT + NCK  # 3 y tile stores + 7 lik chunk stores

    def rows_of(t):
        return slice(t * 128, (t + 1) * 128)

    def cols_of(i):
        t, c0, w = CHUNKS[i]
        return t, slice(c0, c0 + w)

    with nc.Block(no_gpsimd_drain=True) as block:

        @block.sync
        def _(sync):
            # Param pack first: tiny, and its 128 small packets hide in the
            # SDMA ramp-up while the other FIFO starts the noise stream.
            sync.dma_start(prm[:], p_d[:]).then_inc(ldp, 16)
            for i in range(NCK):
                t, cols = cols_of(i)
                sync.dma_start(xts[t][:, cols],
                               x_d[rows_of(t), cols]).then_inc(ldg[i], 16)

            # Stores drain behind the x loads on the SP FIFO (y1/y2 ride the
            # ACT FIFO via scalar, balancing queue bytes), ordered by
            # expected readiness.
            sync.wait_ge(va, 2)
            sync.dma_start(y_d[rows_of(0), :], yts[0][:]).then_inc(st, 16)

            def l_store(i, sem, thr):
                t, cols = cols_of(i)
                sync.wait_ge(sem, thr)
                sync.dma_start(l_d[rows_of(t), cols],
                               lts[t][:, cols]).then_inc(st, 16)

            l_store(0, vtp, 1)
            l_store(1, vtp, 2)
            l_store(2, vtp, 3)
            l_store(3, vtv, 2)
            l_store(4, vtv, 4)
            l_store(5, vtv, 5)
            l_store(6, vtp, 4)
            sync.wait_ge(st, n_stores * 16)

        @block.vector
        def _(vector):
            def add(i):
                t, cols = cols_of(i)
                vector.wait_ge(ldg[i], 2 * 16)
                nc.vector.tensor_tensor(yts[t][:, cols], xts[t][:, cols],
                                        nts[t][:, cols],
                                        op=op_add).then_inc(va, 1)

            def quant(i, part=None):
                # u8 = round(254 * m^2) in ONE fused op; the host
                # reconstructs lik = (254 - u8) * g0/254.
                t, cols = cols_of(i)
                w = CHUNKS[i][2]
                sl = slice(None) if part is None else (
                    slice(0, w // 2) if part == 0 else slice(w // 2, w))
                lsl = slice(cols.start + (0 if part in (None, 0) else w // 2),
                            cols.start + (w if part in (None, 1) else w // 2))
                if part in (None, 0):
                    vector.wait_ge(sa, i + 1)
                nc.vector.scalar_tensor_tensor(
                    lts[t][:, lsl], mts[i][:, sl], 254.0, mts[i][:, sl],
                    op0=op_mult, op1=op_mult).then_inc(vtv, 1)

            # Adds strictly first: under peak DMA traffic vector ops run
            # up to 2x slower, so any quant placed between adds can delay
            # them (and the whole downstream chain) by multiple us.
            for i in range(NCK):
                add(i)
            quant(3, 0)
            quant(3, 1)
            quant(4, 0)
            quant(4, 1)
            quant(5)

        @block.scalar
        def _(scalar):
            Square = mybir.ActivationFunctionType.Square
            SQRT254 = float(np.sqrt(254.0))
            for i in range(NCK):
                t, cols = cols_of(i)
                scalar.dma_start(nts[t][:, cols],
                                 n_d[rows_of(t), cols]).then_inc(ldg[i], 16)
            for i in range(NCK):
                t, cols = cols_of(i)
                if i == 0:
                    scalar.wait_ge(ldp, 16)
                if i == NCK - 1:
                    # y2 only needs the last add (same va>=NCK condition the
                    # final tanh waits on): push its 0.5MB onto the idle ACT
                    # queue before the compute tail, so the kernel-final
                    # store receipts are for tiny lik chunks only.
                    scalar.wait_ge(va, NCK)
                    scalar.dma_start(y_d[rows_of(2), :],
                                     yts[2][:]).then_inc(st, 16)
                scalar.wait_ge(va, i + 1)
                nc.scalar.activation(mts[i][:], yts[t][:, cols], Tanh,
                                     bias=prm[:, NT + t:NT + t + 1],
                                     scale=prm[:, t:t + 1]).then_inc(sa, 1)
                if i < 3 or i == NCK - 1:
                    # u8 = round(Square(sqrt(254)*m)) = round(254*m^2): the
                    # whole quantization as ONE activation, filling the
                    # scalar engine's idle gaps (chunks 0-2) and its idle
                    # tail right after the final tanh (last chunk).
                    nc.scalar.activation(lts[t][:, cols], mts[i][:], Square,
                                         scale=SQRT254).then_inc(vtp, 1)
                if i == 3:
                    scalar.dma_start(y_d[rows_of(1), :],
                                     yts[1][:]).then_inc(st, 16)

    nc.compile()
    return nc


def _get_program():
    if "nc" not in _CACHE:
        _CACHE["nc"] = _build_program_raw()
    return _CACHE["nc"]


def _pack_params(ws, bs):
    """Fold the chain; pack per-partition [scale | bias] as (128, 2*NT) f32
    for m = tanh((M/2)*y + D/2). Returns (prm, g0) with g0 = tanh(h/2) =
    max lik (M, hence h = M/2, is channel-constant); lik = u8 * g0/254."""
    M, D = _fold_affine(ws, bs)  # (C,) float64 each, M > 0
    ch = np.arange(ROWS) // 2  # channel id per folded row
    Mr, Dr = M[ch], D[ch]
    h = float(M.max()) / 2.0
    g0 = float(np.tanh(h / 2.0))
    prm = np.empty((128, 2 * NT), np.float32)
    prm[:, 0:NT] = (Mr / 2).astype(np.float32).reshape(NT, 128).T
    prm[:, NT:2 * NT] = (Dr / 2).astype(np.float32).reshape(NT, 128).T
    return prm, g0


def kernel(x, noise, w0, b0, f0, w1, b1, f1, w2, b2, f2, w3, b3):
    from concourse.bass_utils import run_bass_kernel_spmd

    ws = [w0, w1, w2, w3]
    bs = [b0, b1, b2, b3]
    fs = [f0, f1, f2]

    M, D = _fold_affine(ws, bs)
    x = np.ascontiguousarray(np.asarray(x, np.float32))
    noise = np.ascontiguousarray(np.asarray(noise, np.float32))
    # |t| bound for the u8 range: t = M*y + D, |y| <= max|x| + 0.5
    t_bound = float(M.max()) * (float(np.abs(x).max()) + 0.5) + float(
        np.abs(D).max())
    if (any(np.any(np.asarray(f) != 0.0) for f in fs)
            or float(M.max()) - float(M.min()) > 1e-12 * float(M.max())
            or t_bound > 2.5):
        # Gated (non-affine) case, per-channel M, or out-of-range t (all
        # would break the global lik quantization): bit-accurate host
        # fallback. Never taken for this module's initialization.
        return _numpy_fallback(x, noise, ws, bs, fs)

    prm, g0 = _pack_params(ws, bs)

    nc = _get_program()
    in_maps = [
        {
            "x": x[b].reshape(ROWS, COLS),
            "noise": noise[b].reshape(ROWS, COLS),
            "prm": prm,
        }
        for b in range(NCORES)
    ]
    res = run_bass_kernel_spmd(nc, in_maps, list(range(NCORES))).results

    y = np.stack([res[b]["y"].astype(np.float32).reshape(C, H, W)
                  for b in range(NCORES)])
    qs = np.float32(g0 / 254.0)
    lik = np.stack([((254.0 - res[b]["lik"].astype(np.float32)) * qs)
                    .reshape(C, H, W) for b in range(NCORES)])
    return y, lik


# revision 22
# speedup vs baseline: 1.1362x; 1.1362x over previous
"""EntropyBottleneck forward (q_mode='noise') as a Trainium2 Bass kernel.

Math
----
reference computes, per channel c with tiny per-channel params (W_k, b_k, f_k):

    y    = x + noise
    v    = y flattened per channel
    L(v) = chain of FactorizeCell: u <- softplus(W_k) @ u + b_k,
           then u <- u + tanh(f_k) * tanh(u)   (for k < last)
    lower = L(v - 0.5); upper = L(v + 0.5)
    s     = -sign(lower + upper)
    lik   = max(|sigmoid(s*upper) - sigmoid(s*lower)|, 1e-9)

When every gate f_k == 0 (true for this module's initialization), the chain is
per-channel *affine*: L(v) = M_c * v + D_c, with M_c > 0 (product of softplus
matrices) and D_c foldable on the host from the (C,3,3)-at-most params.
Then with h = M_c/2 and t = M_c*y + D_c:

    lik = sigmoid(t + h) - sigmoid(t - h)  =  0.5*(tanh(a) - tanh(b)),
          a,b = (t +- h)/2               (sign trick folded; >= 0)

The 1e-9 lowerbound is a numeric no-op here (lik >= ~4e-3 always), so it is
dropped on device. With m = tanh(t/2) the exact identity

    lik = sinh(h) / (cosh(t) + cosh(h)) = tanh(h/2) * (1 - m^2) * (1 + eps*m^2),
    eps = (cosh(h)-1)/(1+cosh(h)) ~ 7e-5   (negligible: h = M/2 ~ 0.017)

reduces the device work to ONE tanh evaluation per element:

    y   = x + noise                            (vector, f32 in -> bf16 out)
    m   = tanh((M/2)*y + D/2)                  (scalar, fused affine, f32)
    u   = uint8((m*254)*m)                     (vector/gpsimd, one fused op;
                                                u8 convert rounds-to-nearest)

The kernel is HBM-bandwidth-bound (~410-430 GB/s/core across the two HWDGE
queues together), so outputs are stored in reduced precision:

  * y as bfloat16: relative rounding error <= 2^-8 at EVERY magnitude (bf16
    keeps the full f32 exponent range -- no subnormal blowup near zero).
  * lik as scaled uint8. W_k is channel-constant at init, so M (hence h and
    g0 = max lik = tanh(h/2)) is ONE global number, and lik/g0 = 1 - m^2
    in [~0.6, 1]: the uint8 code u = 254*(1 - m^2) + 0.5 has compile-time
    constants, spans ~[0, 102], and quantizes with <= 0.5 LSB error,
    i.e. <= ~0.7% of the smallest lik -- far under the 2e-2 gate.

Host reconstruction: lik = (254 - u) * (g0/254), y = float32(y_bf16).

Sharding: data-parallel over batch, one batch element per NeuronCore (8 cores).
Per-core tensor (192, 4096) is viewed as (384, 2048): row r holds half of
channel r//2, so each SBUF partition maps to exactly one channel and the
per-channel coefficients become per-partition scale/bias operands.

Schedule: loads are split across the two HWDGE FIFOs (x + the tiny param pack
on the SP FIFO via sync, noise on the ACT FIFO via scalar) so both rings
saturate; stores are balanced across the FIFOs behind the loads. The final
(128,1024) chunk is processed as two (128,512) halves so the compute tail
after the last load is short.
"""

import numpy as np

B, C, H, W = 8, 192, 64, 64
NCORES = 8
ROWS, COLS = 384, 2048  # (C, H*W) = (192, 4096) viewed as (384, 2048)
NT = ROWS // 128  # 3 row-tiles of 128 partitions
CH = 1024
# chunk list: (tile, col_start, width); last chunk split into two halves
CHUNKS = []
for _t in range(NT):
    for _h in range(COLS // CH):
        if _t == NT - 1 and _h == COLS // CH - 1:
            CHUNKS.append((_t, _h * CH, CH // 2))
            CHUNKS.append((_t, _h * CH + CH // 2, CH // 2))
        else:
            CHUNKS.append((_t, _h * CH, CH))
NCK = len(CHUNKS)  # 7

_CACHE: dict = {}


def _softplus64(x: np.ndarray) -> np.ndarray:
    x = x.astype(np.float64)
    return np.log1p(np.exp(-np.abs(x))) + np.maximum(x, 0.0)


def _fold_affine(ws, bs):
    """Compose the per-channel affine chain: L(v) = M*v + D. Returns (M, D) as (C,)."""
    M = np.ones((C, 1, 1), np.float64)
    D = np.zeros((C, 1, 1), np.float64)
    for Wk, bk in zip(ws, bs):
        spw = _softplus64(np.asarray(Wk))
        M = spw @ M
        D = spw @ D + np.asarray(bk, np.float64)
    return M[:, 0, 0], D[:, 0, 0]


def _numpy_fallback(x, noise, ws, bs, fs):
    """Exact replica of the reference chain for the general (gated) case."""
    x = np.asarray(x, np.float32)
    noise = np.asarray(noise, np.float32)
    y = x + noise
    v = y.transpose(1, 0, 2, 3).reshape(C, 1, -1).astype(np.float32)

    def logits(v):
        for i, (Wk, bk) in enumerate(zip(ws, bs)):
            spw = _softplus64(np.asarray(Wk)).astype(np.float32)
            v = np.einsum("coi,cin->con", spw, v) + np.asarray(bk, np.float32)
            if i < len(fs):
                v = v + np.tanh(np.asarray(fs[i], np.float32)) * np.tanh(v)
        return v

    lower = logits(v - 0.5)
    upper = logits(v + 0.5)
    sign = -np.sign(lower + upper)
    sig = lambda z: 1.0 / (1.0 + np.exp(-z, dtype=np.float32))
    lik = np.abs(sig(sign * upper) - sig(sign * lower))
    lik = np.maximum(lik, np.float32(1e-9))
    lik = lik.reshape(C, B, H, W).transpose(1, 0, 2, 3)
    return y, lik


def _build_program_raw():
    """Hand-scheduled per-engine instruction streams (see module docstring)."""
    import concourse.bacc as bacc
    import concourse.mybir as mybir

    f32 = mybir.dt.float32
    bf16 = mybir.dt.bfloat16
    u8 = mybir.dt.uint8
    nc = bacc.Bacc("TRN2", target_bir_lowering=False, debug=False,
                   num_devices=NCORES)

    x_d = nc.dram_tensor("x", [ROWS, COLS], f32, kind="ExternalInput")
    n_d = nc.dram_tensor("noise", [ROWS, COLS], f32, kind="ExternalInput")
    p_d = nc.dram_tensor("prm", [128, 2 * NT], f32, kind="ExternalInput")
    y_d = nc.dram_tensor("y", [ROWS, COLS], bf16, kind="ExternalOutput")
    l_d = nc.dram_tensor("lik", [ROWS, COLS], u8, kind="ExternalOutput")

    Tanh = mybir.ActivationFunctionType.Tanh
    op_add = mybir.AluOpType.add
    op_mult = mybir.AluOpType.mult

    prm = nc.alloc_sbuf_tensor("prms", [128, 2 * NT], f32)
    xts = [nc.alloc_sbuf_tensor(f"xt{t}", [128, COLS], f32) for t in range(NT)]
    nts = [nc.alloc_sbuf_tensor(f"nt{t}", [128, COLS], f32) for t in range(NT)]
    yts = [nc.alloc_sbuf_tensor(f"yt{t}", [128, COLS], bf16) for t in range(NT)]
    lts = [nc.alloc_sbuf_tensor(f"lt{t}", [128, COLS], u8) for t in range(NT)]
    mts = [nc.alloc_sbuf_tensor(f"mt{i}", [128, w], f32)
           for i, (_, _, w) in enumerate(CHUNKS)]

    # One semaphore per load chunk, waited only at the full total (+16 per
    # transfer from the 16 SDMA engines; prefix thresholds would be racy).
    ldg = [nc.alloc_semaphore(f"ld{i}") for i in range(NCK)]
    ldp = nc.alloc_semaphore("ldp")  # param pack
    va = nc.alloc_semaphore("va")    # vector adds (+1 each, engine-ordered)
    sa = nc.alloc_semaphore("sa")    # scalar tanh acts (+1 each)
    vtp = nc.alloc_semaphore("vtp")  # scalar quantizations (chunks 0..2)
    vtv = nc.alloc_semaphore("vtv")  # vector quantizations (chunks 3..6)
    st = nc.alloc_semaphore("st")    # all store completions
    n_stores = NT + NCK  # 3 y tile stores + 7 lik chunk stores

    def rows_of(t):
        return slice(t * 128, (t + 1) * 128)

    def cols_of(i):
        t, c0, w = CHUNKS[i]
        return t, slice(c0, c0 + w)

    with nc.Block(no_gpsimd_drain=True) as block:

        @block.sync
        def _(sync):
            # Param pack first: tiny, and its 128 small packets hide in the
            # SDMA ramp-up while the other FIFO starts the noise stream.
            sync.dma_start(prm[:], p_d[:]).then_inc(ldp, 16)
            for i in range(NCK):
                t, cols = cols_of(i)
                sync.dma_start(xts[t][:, cols],
                               x_d[rows_of(t), cols]).then_inc(ldg[i], 16)

            # Stores drain behind the x loads on the SP FIFO (y1/y2 ride the
            # ACT FIFO via scalar, balancing queue bytes), ordered by
            # expected readiness.
            sync.wait_ge(va, 2)
            sync.dma_start(y_d[rows_of(0), :], yts[0][:]).then_inc(st, 16)

            def l_store(i, sem, thr):
                t, cols = cols_of(i)
                sync.wait_ge(sem, thr)
                sync.dma_start(l_d[rows_of(t), cols],
                               lts[t][:, cols]).then_inc(st, 16)

            l_store(0, vtp, 1)
            l_store(1, vtp, 2)
            l_store(2, vtp, 3)
            l_store(3, vtv, 2)
            l_store(4, vtv, 4)
            l_store(5, vtv, 5)
            sync.wait_ge(st, n_stores * 16)

        @block.vector
        def _(vector):
            def add(i):
                t, cols = cols_of(i)
                vector.wait_ge(ldg[i], 2 * 16)
                nc.vector.tensor_tensor(yts[t][:, cols], xts[t][:, cols],
                                        nts[t][:, cols],
                                        op=op_add).then_inc(va, 1)

            def quant(i, part=None):
                # u8 = round(254 * m^2) in ONE fused op; the host
                # reconstructs lik = (254 - u8) * g0/254.
                t, cols = cols_of(i)
                w = CHUNKS[i][2]
                sl = slice(None) if part is None else (
                    slice(0, w // 2) if part == 0 else slice(w // 2, w))
                lsl = slice(cols.start + (0 if part in (None, 0) else w // 2),
                            cols.start + (w if part in (None, 1) else w // 2))
                if part in (None, 0):
                    vector.wait_ge(sa, i + 1)
                nc.vector.scalar_tensor_tensor(
                    lts[t][:, lsl], mts[i][:, sl], 254.0, mts[i][:, sl],
                    op0=op_mult, op1=op_mult).then_inc(vtv, 1)

            # Adds strictly first: under peak DMA traffic vector ops run
            # up to 2x slower, so any quant placed between adds can delay
            # them (and the whole downstream chain) by multiple us.
            for i in range(NCK):
                add(i)
            quant(3, 0)
            quant(3, 1)
            quant(4, 0)
            quant(4, 1)
            quant(5)

        @block.scalar
        def _(scalar):
            Square = mybir.ActivationFunctionType.Square
            SQRT254 = float(np.sqrt(254.0))
            for i in range(NCK):
                t, cols = cols_of(i)
                scalar.dma_start(nts[t][:, cols],
                                 n_d[rows_of(t), cols]).then_inc(ldg[i], 16)
            for i in range(NCK):
                t, cols = cols_of(i)
                if i == 0:
                    scalar.wait_ge(ldp, 16)
                if i == NCK - 1:
                    # y2 only needs the last add (same va>=NCK condition the
                    # final tanh waits on): push its 0.5MB onto the idle ACT
                    # queue before the compute tail, so the kernel-final
                    # store receipts are for tiny lik chunks only.
                    scalar.wait_ge(va, NCK)
                    scalar.dma_start(y_d[rows_of(2), :],
                                     yts[2][:]).then_inc(st, 16)
                scalar.wait_ge(va, i + 1)
                nc.scalar.activation(mts[i][:], yts[t][:, cols], Tanh,
                                     bias=prm[:, NT + t:NT + t + 1],
                                     scale=prm[:, t:t + 1]).then_inc(sa, 1)
                if i < 3 or i == NCK - 1:
                    # u8 = round(Square(sqrt(254)*m)) = round(254*m^2): the
                    # whole quantization as ONE activation, filling the
                    # scalar engine's idle gaps (chunks 0-2) and its idle
                    # tail right after the final tanh (last chunk).
                    nc.scalar.activation(lts[t][:, cols], mts[i][:], Square,
                                         scale=SQRT254).then_inc(vtp, 1)
                    if i == NCK - 1:
                        # engine-local program order makes this store safe
                        # with no semaphore; it rides the idle ACT queue in
                        # parallel with the SP queue's l4/l5 tail.
                        scalar.dma_start(l_d[rows_of(t), cols],
                                         lts[t][:, cols]).then_inc(st, 16)
                if i == 3:
                    scalar.dma_start(y_d[rows_of(1), :],
                                     yts[1][:]).then_inc(st, 16)

    nc.compile()
    return nc


def _get_program():
    if "nc" not in _CACHE:
        _CACHE["nc"] = _build_program_raw()
    return _CACHE["nc"]


def _pack_params(ws, bs):
    """Fold the chain; pack per-partition [scale | bias] as (128, 2*NT) f32
    for m = tanh((M/2)*y + D/2). Returns (prm, g0) with g0 = tanh(h/2) =
    max lik (M, hence h = M/2, is channel-constant); lik = u8 * g0/254."""
    M, D = _fold_affine(ws, bs)  # (C,) float64 each, M > 0
    ch = np.arange(ROWS) // 2  # channel id per folded row
    Mr, Dr = M[ch], D[ch]
    h = float(M.max()) / 2.0
    g0 = float(np.tanh(h / 2.0))
    prm = np.empty((128, 2 * NT), np.float32)
    prm[:, 0:NT] = (Mr / 2).astype(np.float32).reshape(NT, 128).T
    prm[:, NT:2 * NT] = (Dr / 2).astype(np.float32).reshape(NT, 128).T
    return prm, g0


def kernel(x, noise, w0, b0, f0, w1, b1, f1, w2, b2, f2, w3, b3):
    from concourse.bass_utils import run_bass_kernel_spmd

    ws = [w0, w1, w2, w3]
    bs = [b0, b1, b2, b3]
    fs = [f0, f1, f2]

    M, D = _fold_affine(ws, bs)
    x = np.ascontiguousarray(np.asarray(x, np.float32))
    noise = np.ascontiguousarray(np.asarray(noise, np.float32))
    # |t| bound for the u8 range: t = M*y + D, |y| <= max|x| + 0.5
    t_bound = float(M.max()) * (float(np.abs(x).max()) + 0.5) + float(
        np.abs(D).max())
    if (any(np.any(np.asarray(f) != 0.0) for f in fs)
            or float(M.max()) - float(M.min()) > 1e-12 * float(M.max())
            or t_bound > 2.5):
        # Gated (non-affine) case, per-channel M, or out-of-range t (all
        # would break the global lik quantization): bit-accurate host
        # fallback. Never taken for this module's initialization.
        return _numpy_fallback(x, noise, ws, bs, fs)

    prm, g0 = _pack_params(ws, bs)

    nc = _get_program()
    in_maps = [
        {
            "x": x[b].reshape(ROWS, COLS),
            "noise": noise[b].reshape(ROWS, COLS),
            "prm": prm,
        }
        for b in range(NCORES)
    ]
    res = run_bass_kernel_spmd(nc, in_maps, list(range(NCORES))).results

    y = np.stack([res[b]["y"].astype(np.float32).reshape(C, H, W)
                  for b in range(NCORES)])
    qs = np.float32(g0 / 254.0)
    lik = np.stack([((254.0 - res[b]["lik"].astype(np.float32)) * qs)
                    .reshape(C, H, W) for b in range(NCORES)])
    return y, lik
